# revision 1
# baseline (speedup 1.0000x reference)
"""GAT (3-layer, 4-head) + graph pooling + MLP on 8 Trainium2 NeuronCores.

Sharding: dst-node partitioning. Each core owns N/8 consecutive dst nodes and
all edges pointing into them (edges sorted by dst). Per layer each core builds
gather-table rows [hW | asrc] for its own nodes, an AllGather replicates the
table, then each core processes its edges: dma_gather of source rows,
attention via one-hot indicator matmuls, PSUM-accumulated softmax denominator
+ weighted message sums per 128-dst block. Graph pooling is mask-driven so the
SPMD program is identical across cores (all per-core structure lives in data).
"""

import math
import os
from contextlib import ExitStack

import numpy as np
import ml_dtypes

bf16 = ml_dtypes.bfloat16

N, E, G = 50000, 1600000, 8
IN, H, C = 64, 4, 32
HID = H * C  # 128
L = 3
NC = 8
NPC = N // NC    # 6250
P = 128
NB = (NPC + P - 1) // P   # 49
HALF = 32768
PAD_DL = 255.0


def _roundup(x, m):
    return (x + m - 1) // m * m


# ---------------------------------------------------------------- host prep
def host_prep(x, edge_index, batch):
    src = np.concatenate([np.asarray(edge_index[0]), np.arange(N, dtype=np.int64)])
    dst = np.concatenate([np.asarray(edge_index[1]), np.arange(N, dtype=np.int64)])
    order = np.argsort(dst, kind="stable")
    src = src[order].astype(np.int64)
    dst = dst[order].astype(np.int64)

    A_lists, B_lists = [], []
    maxA = maxB = 0
    for c in range(NC):
        for b in range(NB):
            lo = c * NPC + b * P
            hi = min(c * NPC + (b + 1) * P, (c + 1) * NPC)
            e0, e1 = np.searchsorted(dst, lo), np.searchsorted(dst, hi)
            s = src[e0:e1]
            dl = (dst[e0:e1] - lo).astype(np.int64)
            am = s < HALF
            A_lists.append((s[am].astype(np.int16), dl[am]))
            B_lists.append(((s[~am] - HALF).astype(np.int16), dl[~am]))
            maxA = max(maxA, int(am.sum()))
            maxB = max(maxB, int((~am).sum()))
    A_BLK = max(_roundup(maxA, 128), 128)
    B_BLK = max(_roundup(maxB, 128), 128)
    NCH = (A_BLK + B_BLK) // P

    batch_np = np.asarray(batch).astype(np.int64)
    graph_cnt = np.bincount(batch_np, minlength=G).astype(np.float64)

    percore = []
    for c in range(NC):
        idxA = np.zeros((16, NB * A_BLK // 16), np.int16)
        idxB = np.zeros((16, NB * B_BLK // 16), np.int16)
        dstl = np.full((P, NB * NCH), PAD_DL, np.float32)
        for b in range(NB):
            ia, da = A_lists[c * NB + b]
            ib, db = B_lists[c * NB + b]
            ii = np.arange(len(ia))
            idxA[ii % 16, b * (A_BLK // 16) + ii // 16] = ia
            jj = np.arange(len(ib))
            idxB[jj % 16, b * (B_BLK // 16) + jj // 16] = ib
            dstl[ii % P, b * NCH + ii // P] = da
            pj = A_BLK + jj
            dstl[pj % P, b * NCH + pj // P] = db
        gb = batch_np[c * NPC:(c + 1) * NPC]
        indg = np.zeros((P, NB * G), np.float32)
        # max-pool: 2 slots per block (a 128-node block spans <= 2 graphs)
        maskT2 = np.zeros((P, NB * 2 * P), np.float32)   # [f-part(dummy-bcast), slot*128+nodecol]
        gmask = np.zeros((P, G * 2 * NB), np.float32)
        for b in range(NB):
            rows = gb[b * P: b * P + P]
            for p0, g in enumerate(rows):
                indg[p0, b * G + int(g)] = 1.0
            if len(rows) == 0:
                continue
            cuts = np.flatnonzero(np.diff(rows)) + 1
            bounds = np.concatenate([[0], cuts, [len(rows)]])
            assert len(bounds) <= 3, f"block spans >2 graphs (core {c}, blk {b})"
            for j in range(len(bounds) - 1):
                c0, c1 = int(bounds[j]), int(bounds[j + 1])
                g = int(rows[c0])
                maskT2[:, (b * 2 + j) * P + c0:(b * 2 + j) * P + c1] = 1.0
                gmask[:, g * 2 * NB + b * 2 + j] = 1.0
        percore.append(dict(
            idxA=np.tile(idxA, (8, 1)),
            idxB=np.tile(idxB, (8, 1)),
            dstl=dstl.astype(bf16),
            xT=np.ascontiguousarray(np.asarray(x)[c * NPC:(c + 1) * NPC].T.astype(np.float32)),
            indg=indg, maskT2=maskT2, gmask=gmask,
        ))
    meta = dict(A_BLK=A_BLK, B_BLK=B_BLK, NCH=NCH,
                inv_cnt=(1.0 / np.maximum(graph_cnt, 1.0)).astype(np.float32).reshape(G, 1))
    return percore, meta


def make_consts(Wp, bp, Wl, att_src, att_dst, bconv, W1, b1, W2, b2, W3, b3, meta):
    for nm, v in (("bp", bp), ("bconv", bconv), ("b1", b1), ("b2", b2), ("b3", b3)):
        assert np.abs(np.asarray(v)).max() == 0.0, f"nonzero bias {nm} unsupported"
    AA = np.zeros((L, HID, 2 * H), np.float32)
    for l in range(L):
        for h in range(H):
            AA[l, h * C:(h + 1) * C, h] = np.asarray(att_src)[l, h]
            AA[l, h * C:(h + 1) * C, H + h] = np.asarray(att_dst)[l, h]
    iota = np.tile(np.arange(P, dtype=np.float32), (P, 1)).astype(bf16)
    return dict(
        Wp=np.asarray(Wp, np.float32),
        Wl0=np.asarray(Wl, np.float32)[0], Wl1=np.asarray(Wl, np.float32)[1],
        Wl2=np.asarray(Wl, np.float32)[2],
        AA0=AA[0], AA1=AA[1], AA2=AA[2],
        W1a=np.asarray(W1, np.float32)[:HID], W1b=np.asarray(W1, np.float32)[HID:],
        W2a=np.asarray(W2, np.float32)[:HID], W2b=np.asarray(W2, np.float32)[HID:],
        W3=np.asarray(W3, np.float32),
        iota=iota, Ib=np.eye(P, dtype=bf16), If=np.eye(P, dtype=np.float32),
        inv_cnt=meta["inv_cnt"],
    )


# ---------------------------------------------------------------- numpy model
def numpy_model(inputs):
    percore, meta = host_prep(inputs["x"], inputs["edge_index"], inputs["batch"])
    cons = make_consts(inputs["Wp"], inputs["bp"], inputs["Wl"], inputs["att_src"],
                       inputs["att_dst"], inputs["bconv"], inputs["W1"], inputs["b1"],
                       inputs["W2"], inputs["b2"], inputs["W3"], inputs["b3"], meta)
    A_BLK, B_BLK, NCH = meta["A_BLK"], meta["B_BLK"], meta["NCH"]
    f32 = np.float32
    h_own = [np.maximum(pc["xT"].T @ cons["Wp"], 0.0) for pc in percore]
    Wls = [cons["Wl0"], cons["Wl1"], cons["Wl2"]]
    AAs = [cons["AA0"], cons["AA1"], cons["AA2"]]
    for l in range(L):
        rows = np.zeros((N, 256), bf16)
        adst_own = []
        for c in range(NC):
            hW = (h_own[c] @ Wls[l]).astype(f32)
            st = hW @ AAs[l]
            rows[c * NPC:(c + 1) * NPC, 0:HID] = hW.astype(bf16)
            rows[c * NPC:(c + 1) * NPC, HID:HID + H] = st[:, 0:H].astype(bf16)
            adst_own.append(st[:, H:2 * H].astype(bf16).astype(f32))
        for c in range(NC):
            pc = percore[c]
            hn = np.zeros((NPC, HID), f32)
            for b in range(NB):
                lo, hi = b * P, min(b * P + P, NPC)
                adst_blk = np.zeros((P, H), f32)
                adst_blk[0:hi - lo] = adst_own[c][lo:hi]
                ia = pc["idxA"][:16, b * (A_BLK // 16):(b + 1) * (A_BLK // 16)].T.reshape(-1)
                ib = pc["idxB"][:16, b * (B_BLK // 16):(b + 1) * (B_BLK // 16)].T.reshape(-1)
                Gt = np.concatenate([
                    rows[ia.astype(np.int64)], rows[HALF + ib.astype(np.int64)]
                ]).astype(f32)
                dl = pc["dstl"][:, b * NCH:(b + 1) * NCH].astype(f32)
                out_ps = np.zeros((P, H + HID), f32)
                for ch in range(NCH):
                    Ge = Gt[ch * P:(ch + 1) * P]
                    Ind = (np.arange(P)[None, :] == dl[:, ch:ch + 1]).astype(f32)
                    adst_e = Ind @ adst_blk
                    eatt = Ge[:, HID:HID + H] + adst_e
                    el = np.maximum(eatt, 0.2 * eatt)
                    w = np.exp(el).astype(bf16).astype(f32)
                    msg = (Ge[:, 0:HID] * np.repeat(w, C, 1)).astype(bf16).astype(f32)
                    out_ps += Ind.T @ np.concatenate([w, msg], 1)
                hb = np.maximum(out_ps[:, H:] * np.repeat(1.0 / out_ps[:, 0:H], C, 1), 0.0)
                hb[hi - lo:] = 0.0
                hn[lo:hi] = hb[0:hi - lo]
            h_own[c] = hn
    sums = np.zeros((G, HID), f32)
    mx = np.zeros((HID, G), f32)
    for c in range(NC):
        pc = percore[c]
        hpad = np.zeros((NB * P, HID), f32)
        hpad[:NPC] = h_own[c]
        segmax = np.zeros((HID, 2 * NB), f32)
        for b in range(NB):
            sums += pc["indg"][:, b * G:(b + 1) * G].T @ hpad[b * P:(b + 1) * P]
            hT = hpad[b * P:(b + 1) * P].T  # [HID, 128]
            for j in range(2):
                m = pc["maskT2"][0, (b * 2 + j) * P:(b * 2 + j + 1) * P]
                segmax[:, b * 2 + j] = (hT * m[None, :]).max(1)
        for g in range(G):
            mx[:, g] = np.maximum(mx[:, g],
                                  (segmax * pc["gmask"][0, g * 2 * NB:(g + 1) * 2 * NB][None, :]).max(1))
    p = np.concatenate([sums * meta["inv_cnt"], mx.T], 1)
    o = np.maximum(p @ np.concatenate([cons["W1a"], cons["W1b"]], 0), 0.0)
    o = np.maximum(o @ np.concatenate([cons["W2a"], cons["W2b"]], 0), 0.0)
    return (o @ cons["W3"]).reshape(G)


# ---------------------------------------------------------------- device kernel
def build(ctx: ExitStack, tc, outs, ins, meta):
    import concourse.bass as bass
    import concourse.mybir as mybir
    from concourse.bass import ds

    nc = tc.nc
    A_BLK, B_BLK, NCH = meta["A_BLK"], meta["B_BLK"], meta["NCH"]
    NSC = NCH // 4
    f32, b16, i16 = mybir.dt.float32, mybir.dt.bfloat16, mybir.dt.int16
    AF = mybir.ActivationFunctionType
    OP = mybir.AluOpType

    cpool = ctx.enter_context(tc.tile_pool(name="consts", bufs=1))
    wpool = ctx.enter_context(tc.tile_pool(name="work", bufs=2))
    gpool = ctx.enter_context(tc.tile_pool(name="gather", bufs=3))
    ppool = ctx.enter_context(tc.tile_pool(name="psum", bufs=2, space="PSUM"))
    opool = ctx.enter_context(tc.tile_pool(name="opsum", bufs=2, space="PSUM"))
    tpool = ctx.enter_context(tc.tile_pool(name="tbpsum", bufs=2, space="PSUM"))

    def load_const(name, shape, dtype):
        t = cpool.tile(shape, dtype, tag=f"c_{name}")
        nc.sync.dma_start(out=t[:], in_=ins[name][:])
        return t

    iota = load_const("iota", [P, P], b16)
    Ib = load_const("Ib", [P, P], b16)
    If = load_const("If", [P, P], f32)
    Wp = load_const("Wp", [IN, P], f32)
    Wl = [load_const(f"Wl{l}", [P, P], f32) for l in range(L)]
    AAl = [load_const(f"AA{l}", [P, 2 * H], f32) for l in range(L)]
    W1a = load_const("W1a", [P, 256], f32)
    W1b = load_const("W1b", [P, 256], f32)
    W2a = load_const("W2a", [P, P], f32)
    W2b = load_const("W2b", [P, P], f32)
    W3 = load_const("W3", [P, 1], f32)
    inv_cnt = load_const("inv_cnt", [G, 1], f32)
    idxA = load_const("idxA", [P, NB * A_BLK // 16], i16)
    idxB = load_const("idxB", [P, NB * B_BLK // 16], i16)
    dstl = load_const("dstl", [P, NB * NCH], b16)
    xT = load_const("xT", [IN, NPC], f32)
    indg = load_const("indg", [P, NB * G], f32)
    gmask = load_const("gmask", [P, G * 2 * NB], f32)

    h_own = cpool.tile([P, NB * P], f32, tag="h_own")
    adst_own = cpool.tile([P, NB * H], b16, tag="adst_own")
    nc.vector.memset(adst_own[:], 0)

    own_rows = [nc.dram_tensor(f"own_rows{l}", [NPC, 256], b16) for l in range(L)]
    tables = [nc.dram_tensor(f"table{l}", [N, 256], b16, addr_space="Shared")
              for l in range(L)]
    maskT2_dram = ins["maskT2"]
    pool_sum_in = nc.dram_tensor("pool_sum_in", [G, HID], f32)
    pool_sum_out = nc.dram_tensor("pool_sum_out", [G, HID], f32, addr_space="Shared")
    pool_max_in = nc.dram_tensor("pool_max_in", [HID, G], f32)
    pool_max_out = nc.dram_tensor("pool_max_out", [HID, G], f32, addr_space="Shared")
    groups = [list(range(NC))]

    def table_build(l):
        for b in range(NB):
            nr = min(P, NPC - b * P)
            hT_ps = tpool.tile([P, P], f32, tag="tb_ps")
            nc.tensor.transpose(out=hT_ps[:], in_=h_own[:, ds(b * P, P)], identity=If[:])
            hT = wpool.tile([P, P], f32, tag="hT")
            nc.scalar.activation(func=AF.Copy, out=hT[:], in_=hT_ps[:])
            hWT_ps = tpool.tile([P, P], f32, tag="tb_ps")
            nc.tensor.matmul(out=hWT_ps[:], lhsT=Wl[l][:], rhs=hT[:], start=True, stop=True)
            hWT = wpool.tile([P, P], f32, tag="hWT")
            nc.scalar.activation(func=AF.Copy, out=hWT[:], in_=hWT_ps[:])
            hW_ps = tpool.tile([P, P], f32, tag="tb_ps")
            nc.tensor.transpose(out=hW_ps[:], in_=hWT[:], identity=If[:])
            row = wpool.tile([P, 256], b16, tag="row")
            nc.scalar.activation(func=AF.Copy, out=row[:, 0:HID], in_=hW_ps[:])
            st_ps = tpool.tile([P, 2 * H], f32, tag="tb_ps")
            nc.tensor.matmul(out=st_ps[:], lhsT=hWT[:], rhs=AAl[l][:], start=True, stop=True)
            nc.scalar.activation(func=AF.Copy, out=row[:, HID:HID + H], in_=st_ps[:, 0:H])
            nc.vector.memset(row[:, HID + H:256], 0)
            nc.vector.tensor_copy(out=adst_own[:, ds(b * H, H)], in_=st_ps[:, H:2 * H])
            nc.sync.dma_start(out=own_rows[l][ds(b * P, nr), :], in_=row[0:nr, :])
        nc.gpsimd.collective_compute(
            "AllGather", mybir.AluOpType.bypass, replica_groups=groups,
            ins=[own_rows[l][:]], outs=[tables[l][:]])

    def edge_phase(l):
        for b in range(NB):
            Gt = gpool.tile([P, NCH, 256], b16, tag="G")
            GCH = 1024
            for off in range(0, A_BLK, GCH):
                n = min(GCH, A_BLK - off)
                nc.gpsimd.dma_gather(
                    Gt[:, off // P:(off + n) // P, :], tables[l][:],
                    idxA[:, ds(b * (A_BLK // 16) + off // 16, n // 16)], n, n, 256)
            for off in range(0, B_BLK, GCH):
                n = min(GCH, B_BLK - off)
                nc.gpsimd.dma_gather(
                    Gt[:, (A_BLK + off) // P:(A_BLK + off + n) // P, :], tables[l][HALF:, :],
                    idxB[:, ds(b * (B_BLK // 16) + off // 16, n // 16)], n, n, 256)
            out_ps = opool.tile([P, H + HID], f32, tag="out_ps")
            for s0 in range(0, NCH, 4):
                gsz = min(4, NCH - s0)
                ind = wpool.tile([P, gsz, P], b16, tag="ind")
                indT_ps = ppool.tile([P, gsz * P], b16, tag="indT_ps")
                adst_ps = ppool.tile([P, gsz, H], f32, tag="adst_ps")
                indT = wpool.tile([P, gsz * P], b16, tag="indT")
                for k in range(gsz):
                    ch = s0 + k
                    nc.vector.tensor_tensor(
                        out=ind[:, k, :], in0=iota[:],
                        in1=dstl[:, b * NCH + ch:b * NCH + ch + 1].to_broadcast([P, P]),
                        op=OP.is_equal)
                    nc.tensor.transpose(out=indT_ps[:, ds(k * P, P)], in_=ind[:, k, :],
                                        identity=Ib[:])
                nc.scalar.activation(func=AF.Copy, out=indT[:], in_=indT_ps[:])
                for k in range(gsz):
                    nc.tensor.matmul(out=adst_ps[:, k, :], lhsT=indT[:, ds(k * P, P)],
                                     rhs=adst_own[:, ds(b * H, H)], start=True, stop=True)
                eatt = wpool.tile([P, gsz, H], f32, tag="eatt")
                nc.vector.tensor_tensor(out=eatt[:], in0=Gt[:, ds(s0, gsz), HID:HID + H],
                                        in1=adst_ps[:], op=OP.add)
                lr = wpool.tile([P, gsz, H], f32, tag="lr")
                nc.vector.tensor_scalar(out=lr[:], in0=eatt[:], scalar1=0.2,
                                        scalar2=None, op0=OP.mult)
                nc.vector.tensor_tensor(out=lr[:], in0=lr[:], in1=eatt[:], op=OP.max)
                wm = wpool.tile([P, gsz, H + HID], b16, tag="wm")
                nc.scalar.activation(out=wm[:, :, 0:H], in_=lr[:], func=AF.Exp)
                for k in range(gsz):
                    nc.vector.tensor_tensor(
                        out=wm[:, k, H:H + HID], in0=Gt[:, s0 + k, 0:HID],
                        in1=wm[:, k, 0:H].rearrange("p (h o) -> p h o", o=1).to_broadcast([P, H, C]),
                        op=OP.mult)
                for k in range(gsz):
                    nc.tensor.matmul(out=out_ps[:], lhsT=ind[:, k, :], rhs=wm[:, k, :],
                                     start=(s0 == 0 and k == 0),
                                     stop=(s0 + gsz == NCH and k == gsz - 1))
            rec = wpool.tile([P, H], f32, tag="rec")
            nc.vector.reciprocal(out=rec[:], in_=out_ps[:, 0:H])
            hb = wpool.tile([P, HID], f32, tag="hb")
            nc.vector.tensor_tensor(
                out=hb[:], in0=out_ps[:, H:H + HID],
                in1=rec[:].rearrange("p (h o) -> p h o", o=1).to_broadcast([P, H, C]), op=OP.mult)
            nr = min(P, NPC - b * P)
            if nr < P:
                nc.vector.memset(h_own[:, ds(b * P, P)], 0)
                nc.scalar.activation(out=h_own[0:nr, ds(b * P, P)], in_=hb[0:nr, :],
                                     func=AF.Relu)
            else:
                nc.scalar.activation(out=h_own[:, ds(b * P, P)], in_=hb[:], func=AF.Relu)

    def pooling():
        sum_ps = opool.tile([G, HID], f32, tag="out_ps")
        segmax = cpool.tile([P, 2 * NB], f32, tag="segmax")
        for b in range(NB):
            nc.tensor.matmul(out=sum_ps[:], lhsT=indg[:, ds(b * G, G)],
                             rhs=h_own[:, ds(b * P, P)], start=(b == 0), stop=(b == NB - 1))
            hT_ps = tpool.tile([P, P], f32, tag="tb_ps")
            nc.tensor.transpose(out=hT_ps[:], in_=h_own[:, ds(b * P, P)], identity=If[:])
            hT = wpool.tile([P, P], f32, tag="hT")
            nc.scalar.activation(func=AF.Copy, out=hT[:], in_=hT_ps[:])
            msk = wpool.tile([P, 2, P], f32, tag="msk")
            nc.sync.dma_start(out=msk[:], in_=maskT2_dram[:, ds(b * 2 * P, 2 * P)])
            mm = wpool.tile([P, 2, P], f32, tag="maskmul")
            for j in range(2):
                nc.vector.tensor_tensor(out=mm[:, j, :], in0=hT[:], in1=msk[:, j, :],
                                        op=OP.mult)
            nc.vector.tensor_reduce(out=segmax[:, ds(b * 2, 2)], in_=mm[:],
                                    axis=mybir.AxisListType.X, op=OP.max)
        sum_sb = wpool.tile([G, HID], f32, tag="sum_sb")
        nc.vector.tensor_copy(out=sum_sb[:], in_=sum_ps[:])
        nc.sync.dma_start(out=pool_sum_in[:], in_=sum_sb[:])
        mx = wpool.tile([P, G], f32, tag="mx")
        gm = wpool.tile([P, 2 * NB], f32, tag="gm")
        for g in range(G):
            nc.vector.tensor_tensor(out=gm[:], in0=segmax[:],
                                    in1=gmask[:, ds(g * 2 * NB, 2 * NB)], op=OP.mult)
            nc.vector.tensor_reduce(out=mx[:, g:g + 1], in_=gm[:],
                                    axis=mybir.AxisListType.X, op=OP.max)
        nc.sync.dma_start(out=pool_max_in[:], in_=mx[:])
        nc.gpsimd.collective_compute("AllReduce", mybir.AluOpType.add, replica_groups=groups,
                                     ins=[pool_sum_in[:]], outs=[pool_sum_out[:]])
        nc.gpsimd.collective_compute("AllReduce", mybir.AluOpType.max, replica_groups=groups,
                                     ins=[pool_max_in[:]], outs=[pool_max_out[:]])
        psb = wpool.tile([G, 256], f32, tag="psb")
        tmp = wpool.tile([G, HID], f32, tag="tmp_sum")
        nc.sync.dma_start(out=tmp[:], in_=pool_sum_out[:])
        nc.vector.tensor_scalar(out=psb[:, 0:HID], in0=tmp[:], scalar1=inv_cnt[:],
                                scalar2=None, op0=OP.mult)
        mxr = wpool.tile([P, G], f32, tag="mxr")
        nc.sync.dma_start(out=mxr[:], in_=pool_max_out[:])
        mxT_ps = tpool.tile([G, P], f32, tag="tb_ps")
        nc.tensor.transpose(out=mxT_ps[:], in_=mxr[:], identity=If[:])
        nc.scalar.activation(func=AF.Copy, out=psb[:, HID:256], in_=mxT_ps[:])

        def transpose_sb(src_ap):
            ps = tpool.tile([P, G], f32, tag="tb_ps")
            nc.tensor.transpose(out=ps[:], in_=src_ap, identity=If[0:G, 0:G])
            sb = wpool.tile([P, G], f32, tag="mlp_tsb")
            nc.scalar.activation(func=AF.Copy, out=sb[:], in_=ps[:])
            return sb
        pTa = transpose_sb(psb[:, 0:HID])
        pTb = transpose_sb(psb[:, HID:256])
        o1_ps = tpool.tile([G, 256], f32, tag="tb_ps")
        nc.tensor.matmul(out=o1_ps[:], lhsT=pTa[:], rhs=W1a[:], start=True, stop=False)
        nc.tensor.matmul(out=o1_ps[:], lhsT=pTb[:], rhs=W1b[:], start=False, stop=True)
        o1 = wpool.tile([G, 256], f32, tag="o1")
        nc.scalar.activation(out=o1[:], in_=o1_ps[:], func=AF.Relu)
        o1Ta = transpose_sb(o1[:, 0:P])
        o1Tb = transpose_sb(o1[:, P:256])
        o2_ps = tpool.tile([G, P], f32, tag="tb_ps")
        nc.tensor.matmul(out=o2_ps[:], lhsT=o1Ta[:], rhs=W2a[:], start=True, stop=False)
        nc.tensor.matmul(out=o2_ps[:], lhsT=o1Tb[:], rhs=W2b[:], start=False, stop=True)
        o2 = wpool.tile([G, P], f32, tag="o2")
        nc.scalar.activation(out=o2[:], in_=o2_ps[:], func=AF.Relu)
        o2T = transpose_sb(o2[:])
        o3_ps = tpool.tile([G, 1], f32, tag="tb_ps")
        nc.tensor.matmul(out=o3_ps[:], lhsT=o2T[:], rhs=W3[:], start=True, stop=True)
        res = wpool.tile([G, 1], f32, tag="res")
        nc.vector.tensor_copy(out=res[:], in_=o3_ps[:])
        nc.sync.dma_start(out=outs["out"][:], in_=res[:])

    # layer-0 initial h = relu(x @ Wp)
    for b in range(NB):
        nr = min(P, NPC - b * P)
        h0_ps = tpool.tile([P, P], f32, tag="tb_ps")
        xt = wpool.tile([IN, P], f32, tag="xt")
        if nr < P:
            nc.vector.memset(xt[:], 0)
        nc.vector.tensor_copy(out=xt[:, 0:nr], in_=xT[:, ds(b * P, nr)])
        nc.tensor.matmul(out=h0_ps[:], lhsT=xt[:], rhs=Wp[:], start=True, stop=True)
        if nr < P:
            nc.vector.memset(h_own[:, ds(b * P, P)], 0)
            nc.scalar.activation(out=h_own[0:nr, ds(b * P, P)], in_=h0_ps[0:nr, :],
                                 func=AF.Relu)
        else:
            nc.scalar.activation(out=h_own[:, ds(b * P, P)], in_=h0_ps[:], func=AF.Relu)

    for l in range(L):
        table_build(l)
        edge_phase(l)
    pooling()


# ---------------------------------------------------------------- entry point
def kernel(**inputs) -> np.ndarray:
    percore, meta = host_prep(inputs["x"], inputs["edge_index"], inputs["batch"])
    cons = make_consts(inputs["Wp"], inputs["bp"], inputs["Wl"], inputs["att_src"],
                       inputs["att_dst"], inputs["bconv"], inputs["W1"], inputs["b1"],
                       inputs["W2"], inputs["b2"], inputs["W3"], inputs["b3"], meta)

    from concourse import bacc
    import concourse.tile as tile
    import concourse.mybir as mybir
    from concourse.bass_utils import run_bass_kernel_spmd

    nc = bacc.Bacc(None, target_bir_lowering=False)
    in_maps = []
    for c in range(NC):
        m = dict(cons)
        m.update({k: percore[c][k] for k in
                  ("idxA", "idxB", "dstl", "xT", "indg", "maskT2", "gmask")})
        in_maps.append(m)

    np_dt = {np.dtype(np.float32): mybir.dt.float32, np.dtype(bf16): mybir.dt.bfloat16,
             np.dtype(np.int16): mybir.dt.int16}
    ins_aps = {k: nc.dram_tensor(k, list(v.shape), np_dt[v.dtype], kind="ExternalInput")
               for k, v in in_maps[0].items()}
    out_t = nc.dram_tensor("out", [G, 1], mybir.dt.float32, kind="ExternalOutput")

    with tile.TileContext(nc) as tc:
        with ExitStack() as ctx:
            build(ctx, tc, {"out": out_t}, ins_aps, meta)
    nc.compile()

    import time as _time
    res = run_bass_kernel_spmd(nc, in_maps, list(range(NC)))
    if os.environ.get("KPROF", "") == "1":
        # no NTFF hook in this container: report warm re-execution wall time
        # (upper bound on device time; includes axon RPC + input upload)
        t0 = _time.time()
        res = run_bass_kernel_spmd(nc, in_maps, list(range(NC)))
        print(f"HW exec time: {int((_time.time() - t0) * 1e9)} ns (warm wall upper bound)")
    return np.asarray(res.results[0]["out"]).reshape(G).astype(np.float32)


if __name__ == "__main__":
    import reference
    inputs = {k: np.asarray(v) for k, v in reference.setup_inputs().items()}
    exp = np.asarray(reference.reference(**inputs))
    got = numpy_model(inputs)
    err = np.abs(got - exp).max() / (np.abs(exp).max() + 1e-12)
    print("numpy model rel err:", err)
    print("exp:", exp)
    print("got:", got)



# revision 5
# speedup vs baseline: 4.8038x; 4.8038x over previous
"""GAT (3-layer, 4-head) + graph pooling + MLP on Trainium2.

Single-NeuronCore design (no collectives): the whole graph fits one core's
SBUF/HBM comfortably and the runtime is dominated by fixed costs (compile,
upload), so avoiding the multi-core collective-compilation path and extra
per-core uploads is the fastest configuration by a wide margin.

Algorithm per layer: a table-build pass computes rows [h@W | asrc] for all
nodes into a DRAM table; an edge pass loops over 391 dst blocks of 128 nodes
(hardware For_i loop), dma-gathers source rows for the block's (dst-sorted)
edges, forms per-channel one-hot dst indicators with is_equal, and uses
indicator matmuls for both the adst gather and the PSUM-accumulated
scatter-add of softmax numerator/denominator. Pooling exploits sorted
`batch`: segment mean/max are plain column-range reduces over the transposed
feature tile. MLP is a handful of small matmuls.
"""

import os
import numpy as np
import ml_dtypes
from contextlib import ExitStack

bf16 = ml_dtypes.bfloat16

N, E, G = 50000, 1600000, 8
IN, H, C = 64, 4, 32
HID = H * C  # 128
L = 3
P = 128
NB = (N + P - 1) // P          # 391
NPAD = NB * P                  # 50048
X_CH = 512
NPAD2 = ((NPAD + X_CH - 1) // X_CH) * X_CH   # 50176
HALF = 25024                   # table split (int16 gather indices)
GCH = 1024                     # max rows per dma_gather call


def _roundup(x, m):
    return (x + m - 1) // m * m


# ---------------------------------------------------------------- host prep
def host_prep(x, edge_index, batch):
    src = np.concatenate([np.asarray(edge_index[0]), np.arange(N, dtype=np.int64)])
    dst = np.concatenate([np.asarray(edge_index[1]), np.arange(N, dtype=np.int64)])
    order = np.argsort(dst, kind="stable")
    ss = src[order].astype(np.int64)
    dd = dst[order].astype(np.int64)
    NE = len(ss)

    blk = dd >> 7
    dloc = dd & 127
    isB = ss >= HALF
    cntA = np.bincount(blk[~isB], minlength=NB)
    cntB = np.bincount(blk[isB], minlength=NB)
    A_BLK = max(_roundup(int(cntA.max()), P), P)
    B_BLK = max(_roundup(int(cntB.max()), P), P)
    W = A_BLK + B_BLK
    NCH = W // P

    # rank of each edge within its (block, A/B-group)
    e0 = np.searchsorted(dd, np.arange(NB) * P)          # block start edge idx
    cA = np.cumsum(~isB)
    cB = np.cumsum(isB)
    preA = np.concatenate([[0], cA])[e0[blk]]            # A-count before block
    preB = np.concatenate([[0], cB])[e0[blk]]
    rankA = cA - 1 - preA                                # valid where ~isB
    rankB = cB - 1 - preB
    slot = blk * W + np.where(isB, A_BLK + rankB, rankA)

    vals = np.where(isB, ss - HALF, ss).astype(np.int16)
    idx = np.zeros((16, NB * W // 16), np.int16)
    idx[slot % 16, slot // 16] = vals

    dstl = np.full((P, NB * NCH), 255.0, np.float32)
    dstl[slot % P, blk * NCH + (slot % W) // P] = dloc
    dstl = dstl.astype(bf16)

    xT = np.zeros((IN, NPAD2), bf16)
    xT[:, :N] = np.asarray(x).T

    batch_np = np.asarray(batch).astype(np.int64)
    gs = np.searchsorted(batch_np, np.arange(G + 1))
    cnt = np.diff(gs)
    assert (cnt > 0).all(), "empty graph unsupported"

    meta = dict(A_BLK=A_BLK, B_BLK=B_BLK, NCH=NCH, gs=gs.tolist(),
                inv_cnt=(1.0 / cnt.astype(np.float64)).astype(np.float32))
    percore = dict(idx=idx, dstl=dstl, xT=xT)
    return percore, meta


def make_consts(Wp, bp, Wl, att_src, att_dst, bconv, W1, b1, W2, b2, W3, b3, meta):
    for nm, v in (("bp", bp), ("bconv", bconv), ("b1", b1), ("b2", b2), ("b3", b3)):
        assert np.abs(np.asarray(v)).max() == 0.0, f"nonzero bias {nm} unsupported"
    AA = np.zeros((L, HID, 2 * H), np.float32)
    for l in range(L):
        for h in range(H):
            AA[l, h * C:(h + 1) * C, h] = np.asarray(att_src)[l, h]
            AA[l, h * C:(h + 1) * C, H + h] = np.asarray(att_dst)[l, h]
    Wl_f = np.asarray(Wl, np.float32)
    Bl = np.einsum("lij,ljk->lik", Wl_f, AA)             # [L, HID, 2H]
    iota = np.tile(np.arange(P, dtype=np.float32), (P, 1)).astype(bf16)
    cons = dict(
        Wp=np.asarray(Wp, np.float32).astype(bf16),
        iota=iota, Ib=np.eye(P, dtype=bf16), If=np.eye(P, dtype=np.float32),
        W1a=np.asarray(W1, np.float32)[:HID], W1b=np.asarray(W1, np.float32)[HID:],
        W2a=np.asarray(W2, np.float32)[:HID], W2b=np.asarray(W2, np.float32)[HID:],
        W3=np.asarray(W3, np.float32),
    )
    for l in range(L):
        cons[f"Wl{l}"] = Wl_f[l].astype(bf16)
        cons[f"Bl{l}"] = Bl[l].astype(bf16)
    return cons


# ---------------------------------------------------------------- numpy model
def numpy_model(inputs):
    percore, meta = host_prep(inputs["x"], inputs["edge_index"], inputs["batch"])
    cons = make_consts(inputs["Wp"], inputs["bp"], inputs["Wl"], inputs["att_src"],
                       inputs["att_dst"], inputs["bconv"], inputs["W1"], inputs["b1"],
                       inputs["W2"], inputs["b2"], inputs["W3"], inputs["b3"], meta)
    f32 = np.float32
    A_BLK, B_BLK, NCH = meta["A_BLK"], meta["B_BLK"], meta["NCH"]
    W = A_BLK + B_BLK
    idx, dstl, xT = percore["idx"], percore["dstl"], percore["xT"]

    hT = np.maximum(cons["Wp"].astype(f32).T @ xT.astype(f32), 0.0).astype(bf16)  # [HID, NPAD2]
    for l in range(L):
        Wl_l = cons[f"Wl{l}"].astype(f32)
        Bl_l = cons[f"Bl{l}"].astype(f32)
        table = np.zeros((NPAD, 132), f32)
        adst_all = np.zeros((P, NB * H), bf16)
        for b in range(NB):
            hblk = hT[:, b * P:(b + 1) * P].astype(f32)       # [HID, 128]
            hW = hblk.T @ Wl_l                                # [128, HID]
            st = hblk.T @ Bl_l                                # [128, 2H]
            table[b * P:(b + 1) * P, 0:HID] = hW.astype(bf16).astype(f32)
            table[b * P:(b + 1) * P, HID:HID + H] = st[:, 0:H].astype(bf16).astype(f32)
            adst_all[:, b * H:(b + 1) * H] = st[:, H:2 * H].astype(bf16)
        hT_new = np.zeros_like(hT)
        for b in range(NB):
            out_ps = np.zeros((P, H + HID), f32)
            adst_blk = adst_all[:, b * H:(b + 1) * H].astype(f32)
            for ch in range(NCH):
                # gathered rows for channel ch
                sl = b * W + ch * P + np.arange(P)
                rows_idx = idx[sl % 16, sl // 16].astype(np.int64)
                if ch >= A_BLK // P:
                    rows_idx = rows_idx + HALF
                Ge = table[rows_idx]                          # [128, 132]
                dl = dstl[:, b * NCH + ch].astype(f32)
                Ind = (np.arange(P)[None, :] == dl[:, None]).astype(f32)  # [e, d]
                adst_e = Ind @ adst_blk
                eatt = Ge[:, HID:HID + H] + adst_e
                el = np.maximum(eatt, 0.2 * eatt)
                w = np.exp(el).astype(bf16).astype(f32)
                msg = (Ge[:, 0:HID] * np.repeat(w, C, 1)).astype(bf16).astype(f32)
                out_ps += Ind.T @ np.concatenate([w, msg], 1)
            rec = 1.0 / (out_ps[:, 0:H] + 1e-16)
            hb = out_ps[:, H:] * np.repeat(rec, C, 1)
            hT_new[:, b * P:(b + 1) * P] = np.maximum(hb, 0.0).astype(bf16).T
        hT = hT_new
    gs = meta["gs"]
    meanT = np.zeros((P, G), f32)
    maxT = np.zeros((P, G), f32)
    for g in range(G):
        seg = hT[:, gs[g]:gs[g + 1]].astype(f32)
        meanT[:, g] = seg.sum(1) * meta["inv_cnt"][g]
        maxT[:, g] = seg.max(1)
    o1 = np.maximum(meanT.T @ cons["W1a"] + maxT.T @ cons["W1b"], 0.0)
    o2 = np.maximum(o1 @ np.concatenate([cons["W2a"], cons["W2b"]], 0), 0.0)
    return (o2 @ cons["W3"]).reshape(G)


# ---------------------------------------------------------------- device kernel
def build(ctx: ExitStack, tc, outs, ins, meta):
    import concourse.mybir as mybir
    from concourse.bass import ds

    nc = tc.nc
    A_BLK, B_BLK, NCH = meta["A_BLK"], meta["B_BLK"], meta["NCH"]
    W = A_BLK + B_BLK
    NCHA = A_BLK // P
    gs = meta["gs"]
    inv_cnt = meta["inv_cnt"]
    f32, b16, i16 = mybir.dt.float32, mybir.dt.bfloat16, mybir.dt.int16
    AF = mybir.ActivationFunctionType
    OP = mybir.AluOpType

    cpool = ctx.enter_context(tc.tile_pool(name="consts", bufs=1))
    wpool = ctx.enter_context(tc.tile_pool(name="work", bufs=2))
    gpool = ctx.enter_context(tc.tile_pool(name="gather", bufs=2))
    ppool = ctx.enter_context(tc.tile_pool(name="psum", bufs=2, space="PSUM"))
    apool = ctx.enter_context(tc.tile_pool(name="apsum", bufs=2, space="PSUM"))
    opool = ctx.enter_context(tc.tile_pool(name="opsum", bufs=1, space="PSUM"))
    tpool = ctx.enter_context(tc.tile_pool(name="tbpsum", bufs=2, space="PSUM"))

    def load_const(name, shape, dtype):
        t = cpool.tile(shape, dtype, tag=f"c_{name}", name=f"c_{name}")
        nc.sync.dma_start(out=t[:], in_=ins[name][:])
        return t

    iota = load_const("iota", [P, P], b16)
    Ib = load_const("Ib", [P, P], b16)
    If = load_const("If", [P, P], f32)
    Wp = load_const("Wp", [IN, HID], b16)
    Wl = [load_const(f"Wl{l}", [P, P], b16) for l in range(L)]
    Bl = [load_const(f"Bl{l}", [P, 2 * H], b16) for l in range(L)]
    W1a = load_const("W1a", [P, 256], f32)
    W1b = load_const("W1b", [P, 256], f32)
    W2a = load_const("W2a", [P, P], f32)
    W2b = load_const("W2b", [P, P], f32)
    W3 = load_const("W3", [P, 1], f32)
    dstl = load_const("dstl", [P, NB * NCH], b16)

    hT_all = cpool.tile([P, NPAD2], b16, tag="hT_all", name="hT_all")
    adst_all = cpool.tile([P, NB * H], b16, tag="adst_all", name="adst_all")

    table = nc.dram_tensor("table", [NPAD, 256], b16)
    idx_rep = nc.dram_tensor("idx_rep", [P, NB * W // 16], i16)
    for k in range(8):
        nc.sync.dma_start(out=idx_rep[ds(16 * k, 16), :], in_=ins["idx"][:, :])

    # ---- h0 = relu(x @ Wp), stored transposed [feat, node]
    with tc.For_i(0, NPAD2, X_CH) as o:
        xc = wpool.tile([IN, X_CH], b16, tag="xc")
        nc.sync.dma_start(out=xc[:], in_=ins["xT"][:, ds(o, X_CH)])
        h0_ps = tpool.tile([P, X_CH], f32, tag="tb_ps")
        nc.tensor.matmul(out=h0_ps[:], lhsT=Wp[:], rhs=xc[:], start=True, stop=True)
        nc.scalar.activation(func=AF.Relu, out=hT_all[:, ds(o, X_CH)], in_=h0_ps[:])

    for l in range(L):
        # ---- table build
        with tc.For_i(0, NB, 1) as b:
            hblk = wpool.tile([P, P], b16, tag="hblk")
            nc.vector.tensor_copy(out=hblk[:], in_=hT_all[:, ds(b * P, P)])
            hW_ps = tpool.tile([P, X_CH], f32, tag="tb_ps")
            nc.tensor.matmul(out=hW_ps[:, 0:P], lhsT=hblk[:],
                             rhs=Wl[l][:], start=True, stop=True)
            st_ps = apool.tile([P, 16], f32, tag="small_ps")
            nc.tensor.matmul(out=st_ps[:, 0:2 * H], lhsT=hblk[:],
                             rhs=Bl[l][:], start=True, stop=True)
            row = wpool.tile([P, 136], b16, tag="row")
            nc.scalar.activation(func=AF.Copy, out=row[:, 0:HID], in_=hW_ps[:, 0:P])
            nc.scalar.activation(func=AF.Copy, out=row[:, HID:HID + H],
                                 in_=st_ps[:, 0:H])
            nc.vector.tensor_copy(out=adst_all[:, ds(b * H, H)],
                                  in_=st_ps[:, H:2 * H])
            nc.sync.dma_start(out=table[ds(b * P, P), 0:HID + H],
                              in_=row[:, 0:HID + H])

        # ---- edge phase
        with tc.For_i(0, NB, 1) as b:
            idx_sb = wpool.tile([P, W // 16], i16, tag="idx_sb")
            nc.sync.dma_start(out=idx_sb[:],
                              in_=idx_rep[:, ds(b * (W // 16), W // 16)])
            Gt = gpool.tile([P, NCH, 256], b16, tag="Gt")
            for o in range(0, A_BLK, GCH):
                n = min(GCH, A_BLK - o)
                nc.gpsimd.dma_gather(
                    Gt[:, o // P:(o + n) // P, :], table[:, :],
                    idx_sb[:, ds(o // 16, n // 16)], n, n, 256)
            for o in range(0, B_BLK, GCH):
                n = min(GCH, B_BLK - o)
                nc.gpsimd.dma_gather(
                    Gt[:, (A_BLK + o) // P:(A_BLK + o + n) // P, :],
                    table[HALF:, :],
                    idx_sb[:, ds((A_BLK + o) // 16, n // 16)], n, n, 256)

            out_ps = opool.tile([P, H + HID], f32, tag="out_ps")
            for s0 in range(0, NCH, 4):
                gsz = min(4, NCH - s0)
                sfx = "" if gsz == 4 else f"_r{gsz}"
                ind = wpool.tile([P, gsz, P], b16, tag=f"ind{sfx}")
                nc.vector.tensor_tensor(
                    out=ind[:],
                    in0=iota[:].rearrange("p (k e) -> p k e", k=1)
                        .to_broadcast([P, gsz, P]),
                    in1=dstl[:, ds(b * NCH + s0, gsz)]
                        .rearrange("p (k o) -> p k o", o=1)
                        .to_broadcast([P, gsz, P]),
                    op=OP.is_equal)
                indT_ps = ppool.tile([P, 4 * P], b16, tag="ind_ps")
                for k in range(gsz):
                    nc.tensor.transpose(out=indT_ps[:, ds(k * P, P)],
                                        in_=ind[:, k, :], identity=Ib[:])
                indT = wpool.tile([P, gsz * P], b16, tag=f"indT{sfx}")
                nc.scalar.activation(func=AF.Copy, out=indT[:],
                                     in_=indT_ps[:, 0:gsz * P])
                adst_ps4 = apool.tile([P, 16], f32, tag="small_ps")
                adst_ps = adst_ps4[:].rearrange("p (k h) -> p k h", k=4)[:, 0:gsz, :]
                for k in range(gsz):
                    nc.tensor.matmul(out=adst_ps[:, k, :],
                                     lhsT=indT[:, ds(k * P, P)],
                                     rhs=adst_all[:, ds(b * H, H)],
                                     start=True, stop=True)
                eatt = wpool.tile([P, gsz, H], f32, tag=f"eatt{sfx}")
                nc.vector.tensor_tensor(out=eatt[:],
                                        in0=Gt[:, ds(s0, gsz), HID:HID + H],
                                        in1=adst_ps[:], op=OP.add)
                lr = wpool.tile([P, gsz, H], f32, tag=f"lr{sfx}")
                nc.vector.tensor_scalar(out=lr[:], in0=eatt[:], scalar1=0.2,
                                        scalar2=None, op0=OP.mult)
                nc.vector.tensor_tensor(out=lr[:], in0=lr[:], in1=eatt[:], op=OP.max)
                wm = wpool.tile([P, gsz, H + HID], b16, tag=f"wm{sfx}")
                nc.scalar.activation(out=wm[:, :, 0:H], in_=lr[:], func=AF.Exp)
                nc.vector.tensor_tensor(
                    out=wm[:, :, H:H + HID].rearrange("p k (h c) -> p k h c", h=H),
                    in0=Gt[:, ds(s0, gsz), 0:HID]
                        .rearrange("p k (h c) -> p k h c", h=H),
                    in1=wm[:, :, 0:H].rearrange("p k (h o) -> p k h o", o=1)
                        .to_broadcast([P, gsz, H, C]),
                    op=OP.mult)
                for k in range(gsz):
                    nc.tensor.matmul(out=out_ps[:], lhsT=ind[:, k, :],
                                     rhs=wm[:, k, :],
                                     start=(s0 == 0 and k == 0),
                                     stop=(s0 + gsz == NCH and k == gsz - 1))
            den = wpool.tile([P, H], f32, tag="den")
            nc.vector.tensor_scalar(out=den[:], in0=out_ps[:, 0:H], scalar1=1e-16,
                                    scalar2=None, op0=OP.add)
            rec = wpool.tile([P, H], f32, tag="rec")
            nc.vector.reciprocal(out=rec[:], in_=den[:])
            hb = wpool.tile([P, HID], f32, tag="hb")
            nc.vector.tensor_tensor(
                out=hb[:].rearrange("p (h c) -> p h c", h=H),
                in0=out_ps[:, H:H + HID].rearrange("p (h c) -> p h c", h=H),
                in1=rec[:].rearrange("p (h o) -> p h o", o=1)
                    .to_broadcast([P, H, C]),
                op=OP.mult)
            hbT_ps = tpool.tile([P, X_CH], f32, tag="tb_ps")
            nc.tensor.transpose(out=hbT_ps[:, 0:P], in_=hb[:], identity=If[:])
            nc.scalar.activation(func=AF.Relu, out=hT_all[:, ds(b * P, P)],
                                 in_=hbT_ps[:, 0:P])

    # ---- pooling (batch is sorted: segments are contiguous column ranges)
    meanT = wpool.tile([P, G], f32, tag="meanT")
    maxT = wpool.tile([P, G], f32, tag="maxT")
    for g in range(G):
        s, e = gs[g], gs[g + 1]
        nc.vector.tensor_reduce(out=meanT[:, g:g + 1], in_=hT_all[:, ds(s, e - s)],
                                axis=mybir.AxisListType.X, op=OP.add)
        nc.vector.tensor_scalar(out=meanT[:, g:g + 1], in0=meanT[:, g:g + 1],
                                scalar1=float(inv_cnt[g]), scalar2=None,
                                op0=OP.mult)
        nc.vector.tensor_reduce(out=maxT[:, g:g + 1], in_=hT_all[:, ds(s, e - s)],
                                axis=mybir.AxisListType.X, op=OP.max)

    # ---- MLP
    def transpose_sb(src_ap, n):
        ps = tpool.tile([P, X_CH], f32, tag="tb_ps")
        nc.tensor.transpose(out=ps[0:n, 0:G], in_=src_ap, identity=If[0:G, 0:G])
        sb = wpool.tile([P, G], f32, tag="mlp_t_sb")
        nc.scalar.activation(func=AF.Copy, out=sb[0:n, :], in_=ps[0:n, 0:G])
        return sb

    o1_pst = tpool.tile([P, X_CH], f32, tag="tb_ps")
    o1_ps = o1_pst[0:G, 0:256]
    nc.tensor.matmul(out=o1_ps, lhsT=meanT[:], rhs=W1a[:], start=True, stop=False)
    nc.tensor.matmul(out=o1_ps, lhsT=maxT[:], rhs=W1b[:], start=False, stop=True)
    o1 = wpool.tile([G, 256], f32, tag="o1")
    nc.scalar.activation(out=o1[:], in_=o1_ps, func=AF.Relu)
    o1Ta = transpose_sb(o1[:, 0:P], P)
    o1Tb = transpose_sb(o1[:, P:256], P)
    o2_pst = tpool.tile([P, X_CH], f32, tag="tb_ps")
    o2_ps = o2_pst[0:G, 0:P]
    nc.tensor.matmul(out=o2_ps, lhsT=o1Ta[:], rhs=W2a[:], start=True, stop=False)
    nc.tensor.matmul(out=o2_ps, lhsT=o1Tb[:], rhs=W2b[:], start=False, stop=True)
    o2 = wpool.tile([G, P], f32, tag="o2")
    nc.scalar.activation(out=o2[:], in_=o2_ps, func=AF.Relu)
    o2T = transpose_sb(o2[:], P)
    o3_pst = tpool.tile([P, X_CH], f32, tag="tb_ps")
    o3_ps = o3_pst[0:G, 0:1]
    nc.tensor.matmul(out=o3_ps, lhsT=o2T[:], rhs=W3[:], start=True, stop=True)
    res = wpool.tile([G, 1], f32, tag="res")
    nc.vector.tensor_copy(out=res[:], in_=o3_ps)
    nc.sync.dma_start(out=outs["out"][:], in_=res[:])


# ---------------------------------------------------------------- entry point
def kernel(**inputs) -> np.ndarray:
    percore, meta = host_prep(inputs["x"], inputs["edge_index"], inputs["batch"])
    cons = make_consts(inputs["Wp"], inputs["bp"], inputs["Wl"], inputs["att_src"],
                       inputs["att_dst"], inputs["bconv"], inputs["W1"], inputs["b1"],
                       inputs["W2"], inputs["b2"], inputs["W3"], inputs["b3"], meta)

    from concourse import bacc
    import concourse.tile as tile
    import concourse.mybir as mybir
    from concourse.bass_utils import run_bass_kernel_spmd

    nc = bacc.Bacc(None, target_bir_lowering=False)
    in_map = dict(cons)
    in_map.update(percore)

    np_dt = {np.dtype(np.float32): mybir.dt.float32,
             np.dtype(bf16): mybir.dt.bfloat16,
             np.dtype(np.int16): mybir.dt.int16}
    ins_aps = {k: nc.dram_tensor(k, list(v.shape), np_dt[v.dtype], kind="ExternalInput")
               for k, v in in_map.items()}
    out_t = nc.dram_tensor("out", [G, 1], mybir.dt.float32, kind="ExternalOutput")

    with tile.TileContext(nc) as tc:
        with ExitStack() as ctx:
            build(ctx, tc, {"out": out_t}, ins_aps, meta)
    nc.compile()

    import time as _time
    res = run_bass_kernel_spmd(nc, [in_map], [0])
    if os.environ.get("KPROF", "") == "1":
        t0 = _time.time()
        res = run_bass_kernel_spmd(nc, [in_map], [0])
        print(f"KPROF warm rerun: {time_ns(t0)} ns")
    return np.asarray(res.results[0]["out"]).reshape(G).astype(np.float32)


def time_ns(t0):
    import time as _time
    return int((_time.time() - t0) * 1e9)


if __name__ == "__main__":
    d = np.load("/tmp/_ref_cache.npz")
    exp = d["__exp__"]
    inputs = {k: d[k] for k in d.files if k != "__exp__"}
    got = numpy_model(inputs)
    err = np.abs(got - exp).max() / (np.abs(exp).max() + 1e-12)
    print("numpy model rel err:", err)
    print("exp:", exp)
    print("got:", got)
